# revision 21
# baseline (speedup 1.0000x reference)
"""Trainium2 kernel for nn_DigitConvolutionalModel (dense_cnn).

Model: x[B,784] -> 3x3 valid conv (single channel) -> flatten[676]
       -> Linear(676,200) + ReLU -> Linear(200,10).

The conv is linear, so it is folded into the first Linear on the host:
  flat = x @ C  (C [784,676] sparse conv matrix)
  h1   = relu(flat @ W1.T + b1) = relu(x @ (C @ W1.T) + b1)
so the device computes a plain 784 -> 200 -> 10 MLP. Pure data
parallelism: batch 32768 is split into 8 shards of 4096, one per core;
weights are replicated. Each core receives x pre-transposed ([784,4096],
pixel on the partition/contraction axis) so both matmuls need no
on-device transpose:
  FC1: h1T[200,b] = W1eff[784,200].T @ xT[784,b]   (lhsT = W1eff)
  FC2: outT[10,b] = W2T[200,10].T  @ h1T[200,b]    (lhsT = W2.T)

Schedule (hand-placed, no TileContext):
 - x pieces [<=128, 2048] alternate between the two HWDGE rings (sync /
   scalar) in PE-consumption order so the aggregate ~400 GB/s drain rate
   is reached immediately (a single ring is issue-rate limited at ~620ns
   per dma_start).
 - The 784 contraction is 6x128 + 16. The 16-row tail tile is row-packed:
   its data is replicated on-device to partition offsets 32/64/96 and the
   4 n-blocks of a half run as 4 concurrent row-tiled matmuls (one PSUM
   bank each), turning 4 serial 512-cycle passes into ~1.
 - A short warm-up matmul loop (zeros) un-throttles the PE clock (HAM)
   before the first x piece lands, sized so it never blocks real work.
 - FC1 drains split ACT (m0, relu-with-bias) / DVE (m1); FC2 is
   column-packed 4x concurrent into one reused PSUM bank per half;
   output quarters stream out as soon as their FC2 drains land.
"""

import os
import numpy as np

import concourse.bass as bass
import concourse.bacc as bacc
import concourse.mybir as mybir
from concourse.bass_utils import run_bass_kernel_spmd

import ml_dtypes

N_CORES = 8
B = 32768
BS = B // N_CORES          # 4096 rows per core
IMG = 28
KSZ = 3
OUTW = IMG - KSZ + 1       # 26
NPIX = IMG * IMG           # 784
HID = 200
NCLS = 10

P = 128                    # SBUF partitions
FD = 512                   # matmul free dim (1 PSUM bank of fp32)
NK = 7                     # contraction tiles over 784 = 6*128 + 16
KT = [P] * 6 + [NPIX - 6 * P]
KTAIL = NPIX - 6 * P       # 16
MT = [P, HID - P]          # hid output tiles: 128 + 72
NHALF = 2                  # batch halves per core (PSUM: 2m x 4n = 8 banks)
HB = BS // NHALF           # 2048
NT = HB // FD              # 4 n-tiles of 512 per half
N_WARM = 24                # HAM warm-up matmuls (N=128, ~115ns each cold)


def _ensure_axon_hooks():
    """Provide antenv.axon_hooks if the image lacks it.

    bass_utils' trace path does `from antenv.axon_hooks import
    get_axon_ntff_profile_hook`; on images without that module the import
    crashes instead of degrading. Register a minimal equivalent that drives
    NTFF profiling via the documented C ABI of the loaded axon PJRT plugin
    (axon_start_nrt_profile / axon_stop_nrt_profile), or returns None so
    bass_utils skips tracing gracefully.
    """
    try:
        import antenv.axon_hooks  # noqa: F401

        return
    except ImportError:
        pass
    import sys
    import types
    import ctypes
    import contextlib

    try:
        import antenv
    except ImportError:
        antenv = types.ModuleType("antenv")
        sys.modules["antenv"] = antenv

    mod = types.ModuleType("antenv.axon_hooks")
    state = {"hook": None, "built": False}

    def _build():
        so_path = None
        try:
            with open("/proc/self/maps") as f:
                for line in f:
                    if "libaxon_pjrt.so" in line:
                        so_path = line.split()[-1]
                        break
        except OSError:
            return None
        if so_path is None:
            return None
        lib = ctypes.CDLL(so_path)
        if not hasattr(lib, "axon_start_nrt_profile"):
            return None
        lib.axon_start_nrt_profile.argtypes = [
            ctypes.POINTER(ctypes.c_int64),
            ctypes.c_size_t,
        ]
        lib.axon_start_nrt_profile.restype = ctypes.c_int64
        lib.axon_stop_nrt_profile.argtypes = [ctypes.c_char_p]
        lib.axon_stop_nrt_profile.restype = ctypes.c_int64

        @contextlib.contextmanager
        def _hook(output_dir, device_ids):
            import jax

            jax.devices()
            if device_ids:
                ids = (ctypes.c_int64 * len(device_ids))(*device_ids)
                rc = lib.axon_start_nrt_profile(ids, len(device_ids))
            else:
                rc = lib.axon_start_nrt_profile(None, 0)
            if rc != 0:
                raise RuntimeError(f"axon_start_nrt_profile rc={rc}")
            try:
                yield
            finally:
                n = lib.axon_stop_nrt_profile(str(output_dir).encode())
                if n <= 0:
                    print(f"ntff profile: rc={n} (no profile written)")

        return _hook

    def get_axon_ntff_profile_hook():
        if not state["built"]:
            state["hook"] = _build()
            state["built"] = True
        return state["hook"]

    def set_axon_ntff_profile_hook(hook):
        state["hook"] = hook
        state["built"] = True

    mod.get_axon_ntff_profile_hook = get_axon_ntff_profile_hook
    mod.set_axon_ntff_profile_hook = set_axon_ntff_profile_hook
    sys.modules["antenv.axon_hooks"] = mod
    antenv.axon_hooks = mod


def _dtypes():
    if os.environ.get("KERNEL_FP32"):
        return mybir.dt.float32, np.float32
    return mybir.dt.bfloat16, ml_dtypes.bfloat16


def _build_nc():
    """Hand-scheduled: explicit per-engine streams and semaphores."""
    mm_dt, _ = _dtypes()
    f32 = mybir.dt.float32
    nc = bacc.Bacc(
        "TRN2",
        target_bir_lowering=False,
        debug=False,
        num_devices=N_CORES,
    )

    xT = nc.dram_tensor("xT", [6 * P, BS], mm_dt, kind="ExternalInput")
    # k6 tail (16 rows); replicated on-device to partition offsets 32/64/96
    # so the 4 row-tiled tail matmuls have aligned operands.
    xk6 = nc.dram_tensor("xk6", [KTAIL, BS], mm_dt, kind="ExternalInput")
    w1 = nc.dram_tensor("w1t", [P, NK * HID], mm_dt, kind="ExternalInput")
    w2 = nc.dram_tensor("w2t", [P, 2 * NCLS], mm_dt, kind="ExternalInput")
    b1 = nc.dram_tensor("b1t", [P, 2], f32, kind="ExternalInput")
    b2 = nc.dram_tensor("b2r", [P, 1], f32, kind="ExternalInput")
    outT = nc.dram_tensor("outT", [NCLS, BS], f32, kind="ExternalOutput")

    # SBUF
    xts = [nc.alloc_sbuf_tensor(f"xt{k}", [P, BS], mm_dt).ap() for k in range(NK)]
    w1s = nc.alloc_sbuf_tensor("w1s", [P, NK * HID], mm_dt).ap()
    w2s = nc.alloc_sbuf_tensor("w2s", [P, 2 * NCLS], mm_dt).ap()
    b1s = nc.alloc_sbuf_tensor("b1s", [P, 2], f32).ap()
    b2s = nc.alloc_sbuf_tensor("b2s", [P, 1], f32).ap()
    h1a = nc.alloc_sbuf_tensor("h1a", [MT[0], BS], mm_dt).ap()
    h1b = nc.alloc_sbuf_tensor("h1b", [MT[1], BS], mm_dt).ap()
    ot = nc.alloc_sbuf_tensor("ot", [NCLS, BS], f32).ap()
    warm = nc.alloc_sbuf_tensor("warm", [P, P], mm_dt).ap()

    # PSUM: 2m x NT tensors covering all 8 banks; FC2 reuses ps[0][0/1].
    ps = [
        [nc.alloc_psum_tensor(f"ps_{m}_{n}", [MT[m], FD], f32).ap() for n in range(NT)]
        for m in range(2)
    ]

    # one sem per DMA transfer (completions across a queue are unordered)
    s_x = [[nc.alloc_semaphore(f"s_x_{h}_{k}") for k in range(NK)] for h in range(NHALF)]
    s_x0b = nc.alloc_semaphore("s_x0b")
    s_xr = [nc.alloc_semaphore(f"s_xr_{h}") for h in range(NHALF)]  # k6 replicas
    s_warm = nc.alloc_semaphore("s_warm")
    s_w1a = nc.alloc_semaphore("s_w1a")    # w1 k0 slice
    s_w1b = nc.alloc_semaphore("s_w1b")    # w1 rest
    s_b1 = nc.alloc_semaphore("s_b1")
    s_b2 = nc.alloc_semaphore("s_b2")
    s_w2 = nc.alloc_semaphore("s_w2")
    s_mm = nc.alloc_semaphore("s_mm")      # FC1 bank closers
    s_da = nc.alloc_semaphore("s_da")      # ACT drains (m0)
    s_dv = nc.alloc_semaphore("s_dv")      # DVE drains (m1)
    s_mm2 = nc.alloc_semaphore("s_mm2")    # FC2 strip closers
    s_f2a = nc.alloc_semaphore("s_f2a")    # FC2 ACT drains (odd strips)
    s_f2b = nc.alloc_semaphore("s_f2b")    # FC2 DVE drains (even strips)
    s_out = nc.alloc_semaphore("s_out")
    s_out2 = nc.alloc_semaphore("s_out2")
    all_sems = (
        [s for h in s_x for s in h]
        + [s_x0b] + s_xr
        + [s_warm, s_w1a, s_w1b, s_b1, s_b2, s_w2, s_mm, s_da, s_dv,
           s_mm2, s_f2a, s_f2b, s_out, s_out2]
    )

    # FC1 bank-closer index (1-based s_mm value) for bank (h, m, n).
    # Closers per half: first the m0 k6-pack (n0..n3), then the m1 pack.
    def closer(h, m, n):
        return h * 2 * NT + m * NT + n + 1

    with nc.Block(no_gpsimd_drain=True) as block:

        @block.sync
        def _(sync):
            # h0 pieces (k0 split in two so compute starts earliest; k6 is
            # tiny and its replicas are issued as soon as it lands — the
            # tail pack consumes them only at k-slot 6).
            sync.dma_start(xts[0][:, 0 : HB // 2], xT[0:P, 0 : HB // 2]).then_inc(
                s_x[0][0], 16
            )
            sync.dma_start(
                xts[0][:, HB // 2 : HB], xT[0:P, HB // 2 : HB]
            ).then_inc(s_x0b, 16)
            sync.dma_start(xts[6][0:KTAIL, 0:HB], xk6[:, 0:HB]).then_inc(
                s_x[0][6], 16
            )
            for k in (2, 4):
                sync.dma_start(
                    xts[k][0:P, 0:HB], xT[k * P : (k + 1) * P, 0:HB]
                ).then_inc(s_x[0][k], 16)
            # h0 k6 replicas to partition offsets 32/64/96 (SBUF->SBUF)
            sync.wait_ge(s_x[0][6], 16)
            for j in (1, 2, 3):
                sync.dma_start(
                    xts[6][32 * j : 32 * j + KTAIL, 0:HB], xts[6][0:KTAIL, 0:HB]
                ).then_inc(s_xr[0], 16)
            # h1: sync carries k0, k6(+replicas), k2, k4
            sync.dma_start(xts[0][0:P, HB:BS], xT[0:P, HB:BS]).then_inc(
                s_x[1][0], 16
            )
            sync.dma_start(xts[6][0:KTAIL, HB:BS], xk6[:, HB:BS]).then_inc(
                s_x[1][6], 16
            )
            sync.wait_ge(s_x[1][6], 16)
            for j in (1, 2, 3):
                sync.dma_start(
                    xts[6][32 * j : 32 * j + KTAIL, HB:BS], xts[6][0:KTAIL, HB:BS]
                ).then_inc(s_xr[1], 16)
            for k in (2, 4):
                sync.dma_start(
                    xts[k][0:P, HB:BS], xT[k * P : (k + 1) * P, HB:BS]
                ).then_inc(s_x[1][k], 16)
            # even-strip output blocks (gated by DVE FC2 drains)
            for i, b in enumerate((0, 2, 4, 6)):
                sync.wait_ge(s_f2b, i + 1)
                sync.dma_start(
                    outT[:, b * FD : (b + 1) * FD], ot[0:NCLS, b * FD : (b + 1) * FD]
                ).then_inc(s_out, 16)
            sync.wait_ge(s_out, 64)

        @block.scalar
        def _(scalar):
            # w1 k0 slice first: unblocks the first matmuls. Then yield the
            # HBM channel until the first x piece is in (the ramp is
            # latency-critical; concurrent transfers stretch it).
            scalar.dma_start(w1s[:, 0:HID], w1[:, 0:HID]).then_inc(s_w1a, 16)
            scalar.wait_ge(s_x[0][0], 16)
            scalar.dma_start(
                xts[1][0:P, 0:HB], xT[P : 2 * P, 0:HB]
            ).then_inc(s_x[0][1], 16)
            scalar.dma_start(w1s[:, HID:], w1[:, HID:]).then_inc(s_w1b, 16)
            for k in (3, 5):
                scalar.dma_start(
                    xts[k][0:P, 0:HB], xT[k * P : (k + 1) * P, 0:HB]
                ).then_inc(s_x[0][k], 16)
            scalar.dma_start(w2s[:], w2[:]).then_inc(s_w2, 16)
            scalar.dma_start(b1s[:], b1[:]).then_inc(s_b1, 16)
            scalar.dma_start(b2s[:], b2[:]).then_inc(s_b2, 16)
            for k in (1, 3, 5):
                scalar.dma_start(
                    xts[k][0:P, HB:BS], xT[k * P : (k + 1) * P, HB:BS]
                ).then_inc(s_x[1][k], 16)
            # FC1 m0 drains: relu + bias from PSUM -> h1a (bf16 cast)
            scalar.wait_ge(s_b1, 16)
            for h in range(NHALF):
                for n in range(NT):
                    scalar.wait_ge(s_mm, closer(h, 0, n))
                    c = h * HB + n * FD
                    nc.scalar.activation(
                        h1a[:, c : c + FD],
                        ps[0][n][:],
                        mybir.ActivationFunctionType.Relu,
                        bias=b1s[0 : MT[0], 0:1],
                    ).then_inc(s_da, 1)
            # FC2 drains, odd strips (blocks h*NT+j with j odd)
            scalar.wait_ge(s_b2, 16)
            for h in range(NHALF):
                for j in (1, 3):
                    scalar.wait_ge(s_mm2, h * NT + j + 1)
                    c = (h * NT + j) * FD
                    nc.scalar.activation(
                        ot[:, c : c + FD],
                        ps[0][h][32 * j : 32 * j + NCLS, :],
                        mybir.ActivationFunctionType.Identity,
                        bias=b2s[0:NCLS, :],
                    ).then_inc(s_f2a, 1)
                    # store this block right away (engine-local drain order
                    # guarantees ot block is written)
                    b = h * NT + j
                    scalar.dma_start(
                        outT[:, b * FD : (b + 1) * FD],
                        ot[0:NCLS, b * FD : (b + 1) * FD],
                    ).then_inc(s_out2, 16)
            scalar.wait_ge(s_out2, 64)

        @block.tensor
        def _(tensor):
            # Warm-up: keep the PE active while the first x piece loads so
            # HAM un-throttles to 2.4 GHz ASAP. Short N=128 matmuls so the
            # queue drains quickly once real work is ready.
            tensor.wait_ge(s_warm, 1)
            for _ in range(N_WARM):
                nc.tensor.matmul(
                    ps[0][0][0:P, 0:P], warm[:], warm[:], start=True, stop=True
                )
            tensor.wait_ge(s_w1a, 16)
            for h in range(NHALF):
                c0 = h * HB
                for k in (0, 1, 2, 3, 4, 6, 5):
                    if k == 6:
                        # k6 tail (16 rows): 4 concurrent row-tiled matmuls,
                        # one per n-block, via on-device replicas at offsets
                        # 32/64/96. Second-to-last so the replicas have time
                        # to land; k5 stays the closer so drains pipeline.
                        tensor.wait_ge(s_x[h][6], 16)
                        tensor.wait_ge(s_xr[h], 48)
                        for m in range(2):
                            for j in range(NT):
                                nc.tensor.matmul(
                                    ps[m][j][:],
                                    w1s[
                                        32 * j : 32 * j + KTAIL,
                                        6 * HID + m * P : 6 * HID + m * P + MT[m],
                                    ],
                                    xts[6][
                                        32 * j : 32 * j + KTAIL,
                                        c0 + j * FD : c0 + (j + 1) * FD,
                                    ],
                                    start=False,
                                    stop=False,
                                    tile_position=(32 * j, 0),
                                    skip_group_check=True,
                                )
                        continue
                    if k == 1:
                        tensor.wait_ge(s_w1b, 16)
                    if not (h == 0 and k == 0):
                        tensor.wait_ge(s_x[h][k], 16)
                    if h == 0 and k == 0:
                        mn_order = [(0, 0), (0, 1), (1, 0), (1, 1),
                                    (0, 2), (0, 3), (1, 2), (1, 3)]
                    else:
                        mn_order = [(m, n) for m in range(2) for n in range(NT)]
                    for m, n in mn_order:
                        lhsT = w1s[0:P, k * HID + m * P : k * HID + m * P + MT[m]]
                        if h == 0 and k == 0:
                            if (m, n) == (0, 0):
                                tensor.wait_ge(s_x[0][0], 16)
                            if (m, n) == (0, 2):
                                tensor.wait_ge(s_x0b, 16)
                        if h == 1 and k == 0:
                            # bank reuse: wait for phase-0 drain (WAR)
                            if m == 0:
                                tensor.wait_ge(s_da, n + 1)
                            else:
                                tensor.wait_ge(s_dv, n + 1)
                        mm = nc.tensor.matmul(
                            ps[m][n][:],
                            lhsT,
                            xts[k][0:P, c0 + n * FD : c0 + (n + 1) * FD],
                            start=(k == 0),
                            stop=(k == 5),
                        )
                        if k == 5:
                            mm.then_inc(s_mm, 1)
            # FC2: per half, 4 column-packed concurrent strips into the
            # ps[0][h] bank (which by now holds h1-phase data, drained at
            # s_da >= 5+h); chain K=128 (h1a) then K=72 (h1b).
            tensor.wait_ge(s_w2, 16)
            for h in range(NHALF):
                for j in range(NT):
                    b = h * NT + j
                    # bank free of h1-phase data AND h1a source block drained
                    tensor.wait_ge(s_da, max(NT + h + 1, b + 1))
                    o = ps[0][h][32 * j : 32 * j + NCLS, :]
                    nc.tensor.matmul(
                        o,
                        w2s[0 : MT[0], 0:NCLS],
                        h1a[:, b * FD : (b + 1) * FD],
                        start=True,
                        stop=False,
                        tile_position=(0, 32 * j),
                        skip_group_check=True,
                    )
                for j in range(NT):
                    b = h * NT + j
                    tensor.wait_ge(s_dv, b + 1)  # h1b source block drained
                    o = ps[0][h][32 * j : 32 * j + NCLS, :]
                    nc.tensor.matmul(
                        o,
                        w2s[0 : MT[1], NCLS : 2 * NCLS],
                        h1b[:, b * FD : (b + 1) * FD],
                        start=False,
                        stop=True,
                        tile_position=(0, 32 * j),
                        skip_group_check=True,
                    ).then_inc(s_mm2, 1)

        @block.vector
        def _(vector):
            vector.memset(warm[:], 0.0).then_inc(s_warm, 1)
            vector.wait_ge(s_b1, 16)
            # FC1 m1 drains: (psum + b1) max 0 -> h1b (bf16 cast)
            for h in range(NHALF):
                for n in range(NT):
                    vector.wait_ge(s_mm, closer(h, 1, n))
                    c = h * HB + n * FD
                    nc.vector.tensor_scalar(
                        h1b[:, c : c + FD],
                        ps[1][n][:],
                        b1s[0 : MT[1], 1:2],
                        0.0,
                        mybir.AluOpType.add,
                        mybir.AluOpType.max,
                    ).then_inc(s_dv, 1)
            # FC2 drains, even strips
            vector.wait_ge(s_b2, 16)
            for h in range(NHALF):
                for j in (0, 2):
                    vector.wait_ge(s_mm2, h * NT + j + 1)
                    c = (h * NT + j) * FD
                    nc.vector.tensor_scalar_add(
                        ot[0:NCLS, c : c + FD],
                        ps[0][h][32 * j : 32 * j + NCLS, :],
                        b2s[0:NCLS, :],
                    ).then_inc(s_f2b, 1)

        # After the block-exit all-engine barrier every engine is synced;
        # reset sems so a re-execution of the NEFF starts clean.
        ids = sorted(s.num for s in all_sems)
        if ids == list(range(ids[0], ids[-1] + 1)):
            nc.gpsimd.sem_clear(range(ids[0], ids[-1] + 1))
        else:
            for s in all_sems:
                nc.gpsimd.sem_clear(s)

    nc.compile()
    nc.finalize()
    return nc


def _fold_weights(conv_w, W1):
    """W1eff[784,200] such that x @ W1eff == conv2d_valid(x, conv_w).flat @ W1.T"""
    W1r = W1.reshape(HID, OUTW, OUTW).transpose(1, 2, 0)  # [26,26,200]
    w1e = np.zeros((IMG, IMG, HID), np.float32)
    for di in range(KSZ):
        for dj in range(KSZ):
            w1e[di : di + OUTW, dj : dj + OUTW, :] += conv_w[di, dj] * W1r
    return w1e.reshape(NPIX, HID)


def _prepare_maps(x, conv_w, W1, b1, W2, b2):
    _, np_dt = _dtypes()
    x = np.asarray(x, np.float32)
    conv_w = np.asarray(conv_w, np.float32)
    W1 = np.asarray(W1, np.float32)
    b1 = np.asarray(b1, np.float32)
    W2 = np.asarray(W2, np.float32)
    b2 = np.asarray(b2, np.float32)

    w1e = _fold_weights(conv_w, W1)
    w1t = np.zeros((P, NK * HID), np_dt)
    for k in range(6):
        w1t[:, k * HID : (k + 1) * HID] = w1e[k * P : (k + 1) * P, :].astype(np_dt)
    # k6 tail: replicate the 16 weight rows at partition offsets 0/32/64/96
    # to pair with the row-tiled matmuls.
    tail = w1e[6 * P :, :].astype(np_dt)  # [16, 200]
    for j in range(4):
        w1t[32 * j : 32 * j + KTAIL, 6 * HID : 7 * HID] = tail
    W2T = W2.T  # [200, 10]
    w2t = np.zeros((P, 2 * NCLS), np_dt)
    w2t[: MT[0], 0:NCLS] = W2T[:P].astype(np_dt)
    w2t[: MT[1], NCLS : 2 * NCLS] = W2T[P:].astype(np_dt)
    b1t = np.zeros((P, 2), np.float32)
    b1t[: MT[0], 0] = b1[:P]
    b1t[: MT[1], 1] = b1[P:]
    b2rv = np.zeros((P, 1), np.float32)
    for j in range(4):
        b2rv[32 * j : 32 * j + NCLS, 0] = b2

    xs = x.reshape(N_CORES, BS, NPIX)
    maps = []
    for i in range(N_CORES):
        xTi = xs[i].T.astype(np_dt)  # [784, 4096]
        maps.append(
            {"w1t": w1t, "w2t": w2t, "b1t": b1t, "b2r": b2rv,
             "xT": xTi[: 6 * P], "xk6": xTi[6 * P :].copy()}
        )
    return maps


def _run(inputs, trace=False):
    _ensure_axon_hooks()
    # Build a fresh Bass module per call: re-executing an already-loaded
    # NEFF through this execution path wedges the device, so each call gets
    # its own executable (the NEFF compile cache keeps this cheap).
    nc = _build_nc()
    in_maps = _prepare_maps(**inputs)
    res = run_bass_kernel_spmd(nc, in_maps, list(range(N_CORES)), trace=trace)
    out = np.concatenate([r["outT"].T for r in res.results], axis=0)
    return out, res


def kernel(**inputs):
    out, _ = _run(inputs, trace=False)
    return out
